# revision 30
# baseline (speedup 1.0000x reference)
"""Trainium2 Bass kernel for CropPoolLayer (TF crop_and_resize bilinear + 2x2 maxpool).

Decomposition (host precomputes indices + dense per-ROI bilinear weights):
  per ROI: crops[(i,j), c] = sum_q W[q, (i,j)] * bottom_flat[pt[q], c]
  pooled  = 2x2 max over (i,j)

Device (per core, SPMD over 8 cores, 64 ROIs each, all wire data fp16):
  - per 8-ROI chunk: batched dma_gathers (<=4 columns / 512 idx each; the
    firmware caps one gather at ~1024 idx) of the chunk's points (rows of
    512 ch, 1KB fp16 each) into [128, cols, 512]; ROIs packed at
    64-partition granularity (column = 128 points); chunks mix small and
    large ROIs to balance DMA against compute.
  - transposed matmuls: lhsT = gathered data [pts, 128 c-chunk], rhs =
    weights [pts, 98] -> psum [128(c), 98] so channel is the psum
    partition dim and the (i,j) samples live on the FREE axis.
    Two psum tiles per ROI: psE (even i), psO (odd i), each [128, 4(cc),
    2(dj), 49(pi*7+pj)]. Accumulation groups keep all operands at
    partition base 0 (base-64 accumulation wedges the device).
  - pool: ACT evacuates psE -> sbE; DVE max(psO, sbE) folds di (f32),
    then DVE folds dj on the free axis in fp16 (2x mode) -> [128, 4, 49].
  - output written as [pair, 128(c%128), 2(roi), 4(c//128), 49] fp16,
    un-transposed on host.
"""
import sys

sys.path.insert(0, "/opt/trn_rl_repo")

import numpy as np

POOL = 7
CROP = 14
B, H, W, C = 2, 64, 64, 512
NROI = 512
NCORES = 8
NR = NROI // NCORES            # 64 ROIs per core
SLOTS_PER_CHUNK = 8
NCHUNK = NR // SLOTS_PER_CHUNK  # 8 gather chunks per core

FP16 = np.float16

# m-order permutation: (e, dj, pi*7+pj) -> row (2*pi+e)*14 + (2*pj+dj) of Wfull
_PERM = np.empty((2, 2, 49), np.int64)
for _e in range(2):
    for _dj in range(2):
        for _pi in range(POOL):
            for _pj in range(POOL):
                _PERM[_e, _dj, _pi * 7 + _pj] = (2 * _pi + _e) * 14 + (2 * _pj + _dj)
_PERM_FLAT = _PERM.reshape(-1)


def _grid_geometry(rois, im_info):
    rois = np.asarray(rois, dtype=np.float32)
    im_h = np.float32(im_info[0])
    im_w = np.float32(im_info[1])
    bids = rois[:, 0].astype(np.int32)
    x1 = rois[:, 1] / im_w
    y1 = rois[:, 2] / im_h
    x2 = rois[:, 3] / im_w
    y2 = rois[:, 4] / im_h
    grid = np.arange(CROP, dtype=np.float32)
    ys = y1[:, None] * np.float32(H - 1) + grid[None, :] * ((y2 - y1) * np.float32(H - 1) / np.float32(CROP - 1))[:, None]
    xs = x1[:, None] * np.float32(W - 1) + grid[None, :] * ((x2 - x1) * np.float32(W - 1) / np.float32(CROP - 1))[:, None]
    vy = (ys >= 0) & (ys <= H - 1)
    vx = (xs >= 0) & (xs <= W - 1)
    y0f = np.floor(ys)
    x0f = np.floor(xs)
    ly = (ys - y0f).astype(np.float32)
    lx = (xs - x0f).astype(np.float32)
    y0 = np.clip(y0f, 0, H - 1).astype(np.int32)
    y1i = np.clip(np.ceil(ys), 0, H - 1).astype(np.int32)
    x0 = np.clip(x0f, 0, W - 1).astype(np.int32)
    x1i = np.clip(np.ceil(xs), 0, W - 1).astype(np.int32)
    return dict(bids=bids, ly=ly, lx=lx, y0=y0, y1i=y1i, x0=x0, x1i=x1i, vy=vy, vx=vx)


def _roi_weights(g, n):
    y0 = g["y0"][n]; y1i = g["y1i"][n]; x0 = g["x0"][n]; x1i = g["x1i"][n]
    ly = g["ly"][n]; lx = g["lx"][n]; vy = g["vy"][n]; vx = g["vx"][n]
    rmin = int(min(y0.min(), y1i.min())); rmax = int(max(y0.max(), y1i.max()))
    cmin = int(min(x0.min(), x1i.min())); cmax = int(max(x0.max(), x1i.max()))
    nrows = rmax - rmin + 1; ncols = cmax - cmin + 1
    Wy = np.zeros((CROP, nrows), np.float32)
    Wx = np.zeros((CROP, ncols), np.float32)
    ii = np.arange(CROP)
    np.add.at(Wy, (ii, y0 - rmin), ((1.0 - ly) * vy).astype(np.float32))
    np.add.at(Wy, (ii, y1i - rmin), (ly * vy).astype(np.float32))
    np.add.at(Wx, (ii, x0 - cmin), ((1.0 - lx) * vx).astype(np.float32))
    np.add.at(Wx, (ii, x1i - cmin), (lx * vx).astype(np.float32))
    Wfull = np.einsum("ir,jx->ijrx", Wy, Wx).reshape(CROP * CROP, nrows * ncols)
    return int(g["bids"][n]), rmin, cmin, nrows, ncols, Wfull


def _roi_pts(g):
    pts = np.empty(NROI, np.int64)
    for n in range(NROI):
        y0 = g["y0"][n]; y1i = g["y1i"][n]; x0 = g["x0"][n]; x1i = g["x1i"][n]
        nrows = int(max(y0.max(), y1i.max())) - int(min(y0.min(), y1i.min())) + 1
        ncols = int(max(x0.max(), x1i.max())) - int(min(x0.min(), x1i.min())) + 1
        pts[n] = nrows * ncols
    return pts


def _assign_rois_to_cores(pts):
    """Sort ascending by pts, snake-distribute; per-core slot lists end up
    ascending so cross-core slot maxima stay tight."""
    order = np.argsort(pts, kind="stable")
    cores = [[] for _ in range(NCORES)]
    for rank, n in enumerate(order):
        rnd, pos = divmod(rank, NCORES)
        c = pos if rnd % 2 == 0 else NCORES - 1 - pos
        cores[c].append(int(n))
    return cores


def _make_plan(pts, cores):
    """Shared (cross-core) gather/compute layout.

    Every slot k gets span64[k] = roundup64(max_core pts) point positions,
    packed sequentially; each 8-slot chunk starts on a 128 (column)
    boundary. Returns per-slot q-offsets, per-chunk column ranges and
    per-slot matmul runs (column, partition base, length)."""
    span = [0] * NR
    for k in range(NR):
        m = max(pts[cores[c][k]] for c in range(NCORES))
        span[k] = ((int(m) + 63) // 64) * 64
    # variable chunk sizes: tiny first chunks so the first matmuls start as
    # early as possible, then stride-interleave the remaining sorted slots so
    # each chunk mixes small and large ROIs (balances per-chunk gather DMA
    # against compute)
    chunk_slots = [[t + NCHUNK * j for j in range(SLOTS_PER_CHUNK)]
                   for t in range(NCHUNK)]
    # shorten the drain: move three mid-size slots out of the last chunk
    for dst, rank in zip((4, 5, 6), (31, 39, 47)):
        chunk_slots[NCHUNK - 1].remove(rank)
        chunk_slots[dst].append(rank)
    nchunks = len(chunk_slots)
    # greedy reorder within each chunk to minimize column-align waste: only
    # span-64 slots can occupy a half-open column (offset 64); multi-run
    # slots (span >= 128) must start column-aligned, and those with
    # span % 128 == 64 leave a half-column open that a span-64 slot can fill
    for t in range(nchunks):
        rest = sorted(chunk_slots[t], key=lambda k: span[k])
        if t == 0:
            # ascending order: compute starts on a small single-column slot
            continue
        ordered = []
        o = 0
        while rest:
            pick = None
            if o == 64:
                pick = next((k for k in rest if span[k] == 64), None)
            else:
                half_enders = [k for k in rest if span[k] >= 128 and span[k] % 128 == 64]
                if half_enders and any(span[k] == 64 for k in rest):
                    pick = half_enders[0]
                else:
                    pick = next((k for k in rest if span[k] % 128 == 0), None)
            if pick is None:
                pick = rest[0]
            rest.remove(pick)
            ordered.append(pick)
            if span[pick] + o > 128 and o != 0:
                o = 0
            o = (o + span[pick]) % 128
        chunk_slots[t] = ordered
    emit = [k for cs in chunk_slots for k in cs]
    chunk_of = {}
    for t, cs in enumerate(chunk_slots):
        for k in cs:
            chunk_of[k] = t
    slot_q0 = [0] * NR
    chunk_col0 = [0] * (nchunks + 1)
    q = 0
    for t in range(nchunks):
        assert q % 128 == 0
        chunk_col0[t] = q // 128
        for k in chunk_slots[t]:
            # HW rules: (a) psum accumulation groups (multi-run slots) need
            # base-0 matmul operands -> column-aligned; (b) single-run APs
            # >32 partitions must start at base 0/32/64 -> offsets restricted
            # to {0, 32, 64} within a column.
            o = q % 128
            if span[k] + o > 128:          # would need multiple runs
                if o != 0:
                    q += 128 - o           # column-align
            slot_q0[k] = q
            q += span[k]
        q = ((q + 127) // 128) * 128
    chunk_col0[nchunks] = q // 128
    totq = q
    runs = []
    for k in range(NR):
        rl = []
        s = slot_q0[k] - chunk_col0[chunk_of[k]] * 128
        rem = span[k]
        while rem > 0:
            p0 = s % 128
            g = s // 128
            plen = min(rem, 128 - p0)
            rl.append((g, p0, plen))
            s += plen
            rem -= plen
        runs.append(rl)
    return dict(span=span, slot_q0=slot_q0, chunk_col0=chunk_col0, totq=totq,
                runs=tuple(tuple(r) for r in runs), emit=tuple(emit),
                chunk_slots=tuple(tuple(cs) for cs in chunk_slots))


_NC_CACHE = {}


def _build_bass(plan):
    key = (plan["totq"], tuple(plan["chunk_col0"]), plan["runs"])
    if key in _NC_CACHE:
        return _NC_CACHE[key]
    import concourse.bacc as bacc
    import concourse.mybir as mybir
    from concourse.tile import TileContext

    f32 = mybir.dt.float32
    f16 = mybir.dt.float16
    mmax = mybir.AluOpType.max

    ccol0 = plan["chunk_col0"]
    nchunks = len(ccol0) - 1
    totcols = ccol0[nchunks]
    runs = plan["runs"]

    nc = bacc.Bacc("TRN2", target_bir_lowering=False, debug=False, num_devices=NCORES)
    bottom = nc.dram_tensor("bottom", [B * H * W, C], f16, kind="ExternalInput")
    gidx = nc.dram_tensor("gidx", [128, plan["totq"] // 16], mybir.dt.int16, kind="ExternalInput")
    wts = nc.dram_tensor("w", [128, totcols, 2, 2, 49], f16, kind="ExternalInput")
    out = nc.dram_tensor("out", [NR // 2, 128, 2, 4, 49], f16, kind="ExternalOutput")

    with TileContext(nc) as tc:
        with (
            tc.tile_pool(name="idxp", bufs=3) as idxp,
            tc.tile_pool(name="gp", bufs=3) as gp,
            tc.tile_pool(name="wp", bufs=3) as wp,
            tc.tile_pool(name="ep", bufs=6) as ep,
            tc.tile_pool(name="vp", bufs=6) as vp,
            tc.tile_pool(name="pp", bufs=4) as pp,
            tc.tile_pool(name="psp", bufs=4, space="PSUM") as psp,
        ):
            gts = {}
            wtt = {}

            def issue_loads(t):
                cols = ccol0[t + 1] - ccol0[t]
                it = idxp.tile([128, cols * 8], mybir.dt.int16, tag="idx")
                nc.sync.dma_start(out=it[:], in_=gidx[:, ccol0[t] * 8: ccol0[t + 1] * 8])
                gt = gp.tile([128, cols, C], f16, tag="gt")
                # firmware caps a single gather at 1024 indices; use 4-col
                # pieces so short DMAs can interleave on the DMA engines.
                # chunk 0 leads with a 1-col piece to cut the startup latency
                # to the first matmul.
                j0 = 0
                while j0 < cols:
                    gcols = 1 if (t == 0 and j0 == 0) else min(4, cols - j0)
                    nidx = gcols * 128
                    nc.gpsimd.dma_gather(
                        out_ap=gt[:, j0:j0 + gcols], in_ap=bottom[:],
                        idxs_ap=it[:, j0 * 8: (j0 + gcols) * 8],
                        num_idxs=nidx, num_idxs_reg=nidx,
                        elem_size=C,
                    )
                    j0 += gcols
                wt = wp.tile([128, cols, 2, 2, 49], f16, tag="wt")
                nc.sync.dma_start(out=wt[:], in_=wts[:, ccol0[t]: ccol0[t + 1]])
                gts[t] = gt
                wtt[t] = wt

            issue_loads(0)
            issue_loads(1)
            pair = None
            ei = -1
            for t in range(nchunks):
                if t + 2 < nchunks:
                    issue_loads(t + 2)
                gt = gts.pop(t)
                wt = wtt.pop(t)
                for k in plan["chunk_slots"][t]:
                    ei += 1                         # emission index
                    rl = runs[k]
                    nrun = len(rl)
                    psE = psp.tile([128, 4, 2, 49], f32, tag="psE")
                    psO = psp.tile([128, 4, 2, 49], f32, tag="psO")
                    for cc in range(4):
                        for ri, (g, p0, plen) in enumerate(rl):
                            lhs = gt[p0:p0 + plen, g, cc * 128:(cc + 1) * 128]
                            st = ri == 0
                            sp = ri == nrun - 1
                            nc.tensor.matmul(out=psE[:, cc], lhsT=lhs,
                                             rhs=wt[p0:p0 + plen, g, 0],
                                             start=st, stop=sp)
                            nc.tensor.matmul(out=psO[:, cc], lhsT=lhs,
                                             rhs=wt[p0:p0 + plen, g, 1],
                                             start=st, stop=sp)
                    sbE = ep.tile([128, 4, 2, 49], f32, tag="sbE")
                    nc.scalar.copy(out=sbE[:], in_=psE[:])
                    if ei % 2 == 0:
                        vv = vp.tile([128, 2, 4, 2, 49], f16, tag="v")
                    nc.vector.tensor_tensor(out=vv[:, ei % 2], in0=psO[:], in1=sbE[:], op=mmax)
                    if ei % 2 == 1:
                        # one dj-fold per ROI pair (fp16 2x mode)
                        pair = pp.tile([128, 2, 4, 49], f16, tag="pair")
                        nc.vector.tensor_tensor(out=pair[:], in0=vv[:, :, :, 0],
                                                in1=vv[:, :, :, 1], op=mmax)
                        nc.scalar.dma_start(out=out[ei // 2], in_=pair[:])
    nc.compile()
    _NC_CACHE[key] = nc
    return nc


def _build_core_inputs(g, roi_ids, plan, flat_f16):
    totq = plan["totq"]
    totcols = plan["chunk_col0"][-1]
    gq = np.zeros(totq, np.int16)
    wt = np.zeros((128, totcols, 196), np.float32)
    for k, n in enumerate(roi_ids):
        bid, rmin, cmin, nrows, ncols, Wfull = _roi_weights(g, n)
        pts = nrows * ncols
        rr, xx = np.divmod(np.arange(pts), ncols)
        flat_idx = (bid * (H * W) + (rmin + rr) * W + (cmin + xx)).astype(np.int16)
        qs = plan["slot_q0"][k] + np.arange(pts)
        gq[qs] = flat_idx
        Wsel = Wfull[_PERM_FLAT]                     # [196 (e,dj,pp), pts]
        wt[qs % 128, qs // 128, :] = Wsel.T
    it = np.empty((16, totq // 16), np.int16)
    qpos = np.arange(totq)
    it[qpos % 16, qpos // 16] = gq
    return {
        "bottom": flat_f16,
        "gidx": np.tile(it, (8, 1)),
        "w": wt.reshape(128, totcols, 2, 2, 49).astype(FP16),
    }


def _prepare(bottom, rois, im_info):
    g = _grid_geometry(rois, im_info)
    pts = _roi_pts(g)
    cores = _assign_rois_to_cores(pts)
    plan = _make_plan(pts, cores)
    flat = np.ascontiguousarray(
        np.asarray(bottom, np.float32).reshape(B * H * W, C)).astype(FP16)
    in_maps = [_build_core_inputs(g, cores[c], plan, flat) for c in range(NCORES)]
    return cores, plan, in_maps


def kernel(bottom, rois, im_info):
    from concourse.bass_utils import run_bass_kernel_spmd

    cores, plan, in_maps = _prepare(bottom, rois, im_info)
    nc = _build_bass(plan)
    res = run_bass_kernel_spmd(nc, in_maps, core_ids=list(range(NCORES)))
    out = np.empty((NROI, POOL, POOL, C), np.float32)
    emit = np.array(plan["emit"])
    for c in range(NCORES):
        arr = np.asarray(res.results[c]["out"], dtype=np.float32)
        # [pair, p, r2, cc, pp] -> [pair, r2, pp, cc, p] -> [NR, 49, 512]
        vals = arr.transpose(0, 2, 4, 3, 1).reshape(NR, POOL, POOL, C)
        out[np.array(cores[c])[emit]] = vals
    return out


# revision 31
# speedup vs baseline: 1.0234x; 1.0234x over previous
"""Trainium2 Bass kernel for CropPoolLayer (TF crop_and_resize bilinear + 2x2 maxpool).

Decomposition (host precomputes indices + dense per-ROI bilinear weights):
  per ROI: crops[(i,j), c] = sum_q W[q, (i,j)] * bottom_flat[pt[q], c]
  pooled  = 2x2 max over (i,j)

Device (per core, SPMD over 8 cores, 64 ROIs each, all wire data fp16):
  - per 8-ROI chunk: batched dma_gathers (<=4 columns / 512 idx each; the
    firmware caps one gather at ~1024 idx) of the chunk's points (rows of
    512 ch, 1KB fp16 each) into [128, cols, 512]; ROIs packed at
    64-partition granularity (column = 128 points); chunks mix small and
    large ROIs to balance DMA against compute.
  - transposed matmuls: lhsT = gathered data [pts, 128 c-chunk], rhs =
    weights [pts, 98] -> psum [128(c), 98] so channel is the psum
    partition dim and the (i,j) samples live on the FREE axis.
    Two psum tiles per ROI: psE (even i), psO (odd i), each [128, 4(cc),
    2(dj), 49(pi*7+pj)]. Accumulation groups keep all operands at
    partition base 0 (base-64 accumulation wedges the device).
  - pool: ACT evacuates psE -> sbE; DVE max(psO, sbE) folds di (f32),
    then DVE folds dj on the free axis in fp16 (2x mode) -> [128, 4, 49].
  - output written as [pair, 128(c%128), 2(roi), 4(c//128), 49] fp16,
    un-transposed on host.
"""
import sys

sys.path.insert(0, "/opt/trn_rl_repo")

import numpy as np

POOL = 7
CROP = 14
B, H, W, C = 2, 64, 64, 512
NROI = 512
NCORES = 8
NR = NROI // NCORES            # 64 ROIs per core
SLOTS_PER_CHUNK = 8
NCHUNK = NR // SLOTS_PER_CHUNK  # 8 gather chunks per core

FP16 = np.float16

# m-order permutation: (e, dj, pi*7+pj) -> row (2*pi+e)*14 + (2*pj+dj) of Wfull
_PERM = np.empty((2, 2, 49), np.int64)
for _e in range(2):
    for _dj in range(2):
        for _pi in range(POOL):
            for _pj in range(POOL):
                _PERM[_e, _dj, _pi * 7 + _pj] = (2 * _pi + _e) * 14 + (2 * _pj + _dj)
_PERM_FLAT = _PERM.reshape(-1)


def _grid_geometry(rois, im_info):
    rois = np.asarray(rois, dtype=np.float32)
    im_h = np.float32(im_info[0])
    im_w = np.float32(im_info[1])
    bids = rois[:, 0].astype(np.int32)
    x1 = rois[:, 1] / im_w
    y1 = rois[:, 2] / im_h
    x2 = rois[:, 3] / im_w
    y2 = rois[:, 4] / im_h
    grid = np.arange(CROP, dtype=np.float32)
    ys = y1[:, None] * np.float32(H - 1) + grid[None, :] * ((y2 - y1) * np.float32(H - 1) / np.float32(CROP - 1))[:, None]
    xs = x1[:, None] * np.float32(W - 1) + grid[None, :] * ((x2 - x1) * np.float32(W - 1) / np.float32(CROP - 1))[:, None]
    vy = (ys >= 0) & (ys <= H - 1)
    vx = (xs >= 0) & (xs <= W - 1)
    y0f = np.floor(ys)
    x0f = np.floor(xs)
    ly = (ys - y0f).astype(np.float32)
    lx = (xs - x0f).astype(np.float32)
    y0 = np.clip(y0f, 0, H - 1).astype(np.int32)
    y1i = np.clip(np.ceil(ys), 0, H - 1).astype(np.int32)
    x0 = np.clip(x0f, 0, W - 1).astype(np.int32)
    x1i = np.clip(np.ceil(xs), 0, W - 1).astype(np.int32)
    return dict(bids=bids, ly=ly, lx=lx, y0=y0, y1i=y1i, x0=x0, x1i=x1i, vy=vy, vx=vx)


def _roi_weights(g, n):
    y0 = g["y0"][n]; y1i = g["y1i"][n]; x0 = g["x0"][n]; x1i = g["x1i"][n]
    ly = g["ly"][n]; lx = g["lx"][n]; vy = g["vy"][n]; vx = g["vx"][n]
    rmin = int(min(y0.min(), y1i.min())); rmax = int(max(y0.max(), y1i.max()))
    cmin = int(min(x0.min(), x1i.min())); cmax = int(max(x0.max(), x1i.max()))
    nrows = rmax - rmin + 1; ncols = cmax - cmin + 1
    Wy = np.zeros((CROP, nrows), np.float32)
    Wx = np.zeros((CROP, ncols), np.float32)
    ii = np.arange(CROP)
    np.add.at(Wy, (ii, y0 - rmin), ((1.0 - ly) * vy).astype(np.float32))
    np.add.at(Wy, (ii, y1i - rmin), (ly * vy).astype(np.float32))
    np.add.at(Wx, (ii, x0 - cmin), ((1.0 - lx) * vx).astype(np.float32))
    np.add.at(Wx, (ii, x1i - cmin), (lx * vx).astype(np.float32))
    Wfull = np.einsum("ir,jx->ijrx", Wy, Wx).reshape(CROP * CROP, nrows * ncols)
    return int(g["bids"][n]), rmin, cmin, nrows, ncols, Wfull


def _roi_pts(g):
    pts = np.empty(NROI, np.int64)
    for n in range(NROI):
        y0 = g["y0"][n]; y1i = g["y1i"][n]; x0 = g["x0"][n]; x1i = g["x1i"][n]
        nrows = int(max(y0.max(), y1i.max())) - int(min(y0.min(), y1i.min())) + 1
        ncols = int(max(x0.max(), x1i.max())) - int(min(x0.min(), x1i.min())) + 1
        pts[n] = nrows * ncols
    return pts


def _assign_rois_to_cores(pts):
    """Sort ascending by pts, snake-distribute; per-core slot lists end up
    ascending so cross-core slot maxima stay tight."""
    order = np.argsort(pts, kind="stable")
    cores = [[] for _ in range(NCORES)]
    for rank, n in enumerate(order):
        rnd, pos = divmod(rank, NCORES)
        c = pos if rnd % 2 == 0 else NCORES - 1 - pos
        cores[c].append(int(n))
    return cores


def _make_plan(pts, cores):
    """Shared (cross-core) gather/compute layout.

    Every slot k gets span64[k] = roundup64(max_core pts) point positions,
    packed sequentially; each 8-slot chunk starts on a 128 (column)
    boundary. Returns per-slot q-offsets, per-chunk column ranges and
    per-slot matmul runs (column, partition base, length)."""
    span = [0] * NR
    for k in range(NR):
        m = max(pts[cores[c][k]] for c in range(NCORES))
        span[k] = ((int(m) + 63) // 64) * 64
    # variable chunk sizes: tiny first chunks so the first matmuls start as
    # early as possible, then stride-interleave the remaining sorted slots so
    # each chunk mixes small and large ROIs (balances per-chunk gather DMA
    # against compute)
    chunk_slots = [[t + NCHUNK * j for j in range(SLOTS_PER_CHUNK)]
                   for t in range(NCHUNK)]
    nchunks = len(chunk_slots)
    # greedy reorder within each chunk to minimize column-align waste: only
    # span-64 slots can occupy a half-open column (offset 64); multi-run
    # slots (span >= 128) must start column-aligned, and those with
    # span % 128 == 64 leave a half-column open that a span-64 slot can fill
    for t in range(nchunks):
        rest = sorted(chunk_slots[t], key=lambda k: span[k])
        if t == 0:
            # ascending order: compute starts on a small single-column slot
            continue
        ordered = []
        o = 0
        while rest:
            pick = None
            if o == 64:
                pick = next((k for k in rest if span[k] == 64), None)
            else:
                half_enders = [k for k in rest if span[k] >= 128 and span[k] % 128 == 64]
                if half_enders and any(span[k] == 64 for k in rest):
                    pick = half_enders[0]
                else:
                    pick = next((k for k in rest if span[k] % 128 == 0), None)
            if pick is None:
                pick = rest[0]
            rest.remove(pick)
            ordered.append(pick)
            if span[pick] + o > 128 and o != 0:
                o = 0
            o = (o + span[pick]) % 128
        chunk_slots[t] = ordered
    emit = [k for cs in chunk_slots for k in cs]
    chunk_of = {}
    for t, cs in enumerate(chunk_slots):
        for k in cs:
            chunk_of[k] = t
    slot_q0 = [0] * NR
    chunk_col0 = [0] * (nchunks + 1)
    q = 0
    for t in range(nchunks):
        assert q % 128 == 0
        chunk_col0[t] = q // 128
        for k in chunk_slots[t]:
            # HW rules: (a) psum accumulation groups (multi-run slots) need
            # base-0 matmul operands -> column-aligned; (b) single-run APs
            # >32 partitions must start at base 0/32/64 -> offsets restricted
            # to {0, 32, 64} within a column.
            o = q % 128
            if span[k] + o > 128:          # would need multiple runs
                if o != 0:
                    q += 128 - o           # column-align
            slot_q0[k] = q
            q += span[k]
        q = ((q + 127) // 128) * 128
    chunk_col0[nchunks] = q // 128
    totq = q
    runs = []
    for k in range(NR):
        rl = []
        s = slot_q0[k] - chunk_col0[chunk_of[k]] * 128
        rem = span[k]
        while rem > 0:
            p0 = s % 128
            g = s // 128
            plen = min(rem, 128 - p0)
            rl.append((g, p0, plen))
            s += plen
            rem -= plen
        runs.append(rl)
    return dict(span=span, slot_q0=slot_q0, chunk_col0=chunk_col0, totq=totq,
                runs=tuple(tuple(r) for r in runs), emit=tuple(emit),
                chunk_slots=tuple(tuple(cs) for cs in chunk_slots))


_NC_CACHE = {}


def _build_bass(plan):
    key = (plan["totq"], tuple(plan["chunk_col0"]), plan["runs"])
    if key in _NC_CACHE:
        return _NC_CACHE[key]
    import concourse.bacc as bacc
    import concourse.mybir as mybir
    from concourse.tile import TileContext

    f32 = mybir.dt.float32
    f16 = mybir.dt.float16
    mmax = mybir.AluOpType.max

    ccol0 = plan["chunk_col0"]
    nchunks = len(ccol0) - 1
    totcols = ccol0[nchunks]
    runs = plan["runs"]

    nc = bacc.Bacc("TRN2", target_bir_lowering=False, debug=False, num_devices=NCORES)
    bottom = nc.dram_tensor("bottom", [B * H * W, C], f16, kind="ExternalInput")
    gidx = nc.dram_tensor("gidx", [128, plan["totq"] // 16], mybir.dt.int16, kind="ExternalInput")
    wts = nc.dram_tensor("w", [128, totcols, 2, 2, 49], f16, kind="ExternalInput")
    out = nc.dram_tensor("out", [NR // 2, 128, 2, 4, 49], f16, kind="ExternalOutput")

    with TileContext(nc) as tc:
        with (
            tc.tile_pool(name="idxp", bufs=3) as idxp,
            tc.tile_pool(name="gp", bufs=3) as gp,
            tc.tile_pool(name="wp", bufs=3) as wp,
            tc.tile_pool(name="ep", bufs=6) as ep,
            tc.tile_pool(name="vp", bufs=6) as vp,
            tc.tile_pool(name="pp", bufs=4) as pp,
            tc.tile_pool(name="psp", bufs=4, space="PSUM") as psp,
        ):
            gts = {}
            wtt = {}

            def issue_loads(t):
                cols = ccol0[t + 1] - ccol0[t]
                it = idxp.tile([128, cols * 8], mybir.dt.int16, tag="idx")
                nc.sync.dma_start(out=it[:], in_=gidx[:, ccol0[t] * 8: ccol0[t + 1] * 8])
                gt = gp.tile([128, cols, C], f16, tag="gt")
                # firmware caps a single gather at 1024 indices; use 4-col
                # pieces so short DMAs can interleave on the DMA engines.
                # chunk 0 leads with a 1-col piece to cut the startup latency
                # to the first matmul.
                j0 = 0
                while j0 < cols:
                    gcols = 1 if (t == 0 and j0 == 0) else min(4, cols - j0)
                    nidx = gcols * 128
                    nc.gpsimd.dma_gather(
                        out_ap=gt[:, j0:j0 + gcols], in_ap=bottom[:],
                        idxs_ap=it[:, j0 * 8: (j0 + gcols) * 8],
                        num_idxs=nidx, num_idxs_reg=nidx,
                        elem_size=C,
                    )
                    j0 += gcols
                wt = wp.tile([128, cols, 2, 2, 49], f16, tag="wt")
                nc.sync.dma_start(out=wt[:], in_=wts[:, ccol0[t]: ccol0[t + 1]])
                gts[t] = gt
                wtt[t] = wt

            issue_loads(0)
            issue_loads(1)
            pair = None
            ei = -1
            for t in range(nchunks):
                if t + 2 < nchunks:
                    issue_loads(t + 2)
                gt = gts.pop(t)
                wt = wtt.pop(t)
                for k in plan["chunk_slots"][t]:
                    ei += 1                         # emission index
                    rl = runs[k]
                    nrun = len(rl)
                    psE = psp.tile([128, 4, 2, 49], f32, tag="psE")
                    psO = psp.tile([128, 4, 2, 49], f32, tag="psO")
                    for cc in range(4):
                        for ri, (g, p0, plen) in enumerate(rl):
                            lhs = gt[p0:p0 + plen, g, cc * 128:(cc + 1) * 128]
                            st = ri == 0
                            sp = ri == nrun - 1
                            nc.tensor.matmul(out=psE[:, cc], lhsT=lhs,
                                             rhs=wt[p0:p0 + plen, g, 0],
                                             start=st, stop=sp)
                            nc.tensor.matmul(out=psO[:, cc], lhsT=lhs,
                                             rhs=wt[p0:p0 + plen, g, 1],
                                             start=st, stop=sp)
                    sbE = ep.tile([128, 4, 2, 49], f32, tag="sbE")
                    nc.scalar.copy(out=sbE[:], in_=psE[:])
                    if ei % 2 == 0:
                        vv = vp.tile([128, 2, 4, 2, 49], f16, tag="v")
                    nc.vector.tensor_tensor(out=vv[:, ei % 2], in0=psO[:], in1=sbE[:], op=mmax)
                    if ei % 2 == 1:
                        # one dj-fold per ROI pair (fp16 2x mode)
                        pair = pp.tile([128, 2, 4, 49], f16, tag="pair")
                        nc.vector.tensor_tensor(out=pair[:], in0=vv[:, :, :, 0],
                                                in1=vv[:, :, :, 1], op=mmax)
                        nc.scalar.dma_start(out=out[ei // 2], in_=pair[:])
    nc.compile()
    _NC_CACHE[key] = nc
    return nc


def _build_core_inputs(g, roi_ids, plan, flat_f16):
    totq = plan["totq"]
    totcols = plan["chunk_col0"][-1]
    gq = np.zeros(totq, np.int16)
    wt = np.zeros((128, totcols, 196), np.float32)
    for k, n in enumerate(roi_ids):
        bid, rmin, cmin, nrows, ncols, Wfull = _roi_weights(g, n)
        pts = nrows * ncols
        rr, xx = np.divmod(np.arange(pts), ncols)
        flat_idx = (bid * (H * W) + (rmin + rr) * W + (cmin + xx)).astype(np.int16)
        qs = plan["slot_q0"][k] + np.arange(pts)
        gq[qs] = flat_idx
        Wsel = Wfull[_PERM_FLAT]                     # [196 (e,dj,pp), pts]
        wt[qs % 128, qs // 128, :] = Wsel.T
    it = np.empty((16, totq // 16), np.int16)
    qpos = np.arange(totq)
    it[qpos % 16, qpos // 16] = gq
    return {
        "bottom": flat_f16,
        "gidx": np.tile(it, (8, 1)),
        "w": wt.reshape(128, totcols, 2, 2, 49).astype(FP16),
    }


def _prepare(bottom, rois, im_info):
    g = _grid_geometry(rois, im_info)
    pts = _roi_pts(g)
    cores = _assign_rois_to_cores(pts)
    plan = _make_plan(pts, cores)
    flat = np.ascontiguousarray(
        np.asarray(bottom, np.float32).reshape(B * H * W, C)).astype(FP16)
    in_maps = [_build_core_inputs(g, cores[c], plan, flat) for c in range(NCORES)]
    return cores, plan, in_maps


def kernel(bottom, rois, im_info):
    from concourse.bass_utils import run_bass_kernel_spmd

    cores, plan, in_maps = _prepare(bottom, rois, im_info)
    nc = _build_bass(plan)
    res = run_bass_kernel_spmd(nc, in_maps, core_ids=list(range(NCORES)))
    out = np.empty((NROI, POOL, POOL, C), np.float32)
    emit = np.array(plan["emit"])
    for c in range(NCORES):
        arr = np.asarray(res.results[c]["out"], dtype=np.float32)
        # [pair, p, r2, cc, pp] -> [pair, r2, pp, cc, p] -> [NR, 49, 512]
        vals = arr.transpose(0, 2, 4, 3, 1).reshape(NR, POOL, POOL, C)
        out[np.array(cores[c])[emit]] = vals
    return out
